# revision 11
# baseline (speedup 1.0000x reference)
"""Causal self-attention (B=4, T=2048, C=1024, H=16) on 8 trn2 NeuronCores.

Strategy: tensor-parallel over heads (2 heads/core). Each core:
  phase 1: qkvT[ch, tok] = w_slice.T @ x.T   (fp32r matmuls, full x replicated)
           v transposed on PE to [tok, ch] layout, heads split along free dim
  phase 2: per (batch, head): S^T blocks = k.T-tiles @ q.T-chunks (fp16),
           exp on ACT, causal mask on DVE, PV matmuls accumulate
           y^T[hs+ones, qtok] in PSUM (denominator via ones column)
  phase 3: per batch: reciprocal of denominators (DVE), broadcast via DRAM,
           normalize y, c_proj matmuls (fp16), partial out[tok, C] fp32
Host sums the 8 partial outputs (tensor-parallel all-reduce done on host).

All SBUF/PSUM compute APs use partition offset 0 (nonzero offsets silently
read wrong partitions on this toolchain); head regrouping goes through DRAM.
"""
import sys
sys.path.insert(0, '/opt/trn_rl_repo')

import numpy as np

import concourse.bass as bass
import concourse.bacc as bacc_mod
import concourse.mybir as mybir
import concourse.tile as tile
from concourse.bass_utils import run_bass_kernel_spmd
from concourse.masks import make_identity

F32 = mybir.dt.float32
F32R = mybir.dt.float32r
F16 = mybir.dt.float16

B, T, C, H, HS = 4, 2048, 1024, 16, 64
HPC = H // 8            # heads per core = 2
TOK = B * T             # 8192
NCHUNK = TOK // 512     # 16 tok chunks for qkv
NQC = T // 512          # 4 qtok chunks per batch
SUP = 3                 # ktok blocks per exp supertile (3 psum banks)

_CACHE = {}


def _ap(t, offset, ap):
    return bass.AP(tensor=t.tensor, offset=t.offset + offset, ap=ap)


def build(debug=False):
    nc = bacc_mod.Bacc(target_bir_lowering=False, trn_type="TRN2")

    # fp32r inputs for the qkv projection (host pre-transposed / pre-sliced)
    xt_d = nc.dram_tensor("xt", [C, TOK], F32R, kind="ExternalInput")
    wq_d = nc.dram_tensor("wq", [128, 8, 128], F32R, kind="ExternalInput")
    wk_d = nc.dram_tensor("wk", [128, 8, 128], F32R, kind="ExternalInput")
    wv_d = nc.dram_tensor("wv", [128, 8, 128], F32R, kind="ExternalInput")
    wp_d = nc.dram_tensor("wp", [128, C], F16, kind="ExternalInput")
    out_d = nc.dram_tensor("out", [TOK, C], F32, kind="ExternalOutput")
    if debug:
        dbg_qk = nc.dram_tensor("dbg_qk", [2, 2, 64, TOK], F16, kind="ExternalOutput")
        dbg_ys = nc.dram_tensor("dbg_ys", [B, 2, NQC, 65, 512], F32, kind="ExternalOutput")
        dbg_rc = nc.dram_tensor("dbg_rc", [B, 2 * NQC, 512], F32, kind="ExternalOutput")
        dbg_yn = nc.dram_tensor("dbg_yn", [B, NQC, 128, 512], F16, kind="ExternalOutput")

    with tile.TileContext(nc) as tc:
        with (
            tc.tile_pool(name="singles", bufs=1) as singles,
            tc.tile_pool(name="dram", bufs=1, space="DRAM") as dpool,
        ):
            # persistent DRAM scratch
            qk_dram = dpool.tile([2, 2, 64, TOK], F16)      # [qk, head, ch, tok]
            y_stage = dpool.tile([B, 2, NQC, 65, 512], F32)  # y^T + denom row 64
            yn_dram = dpool.tile([B, NQC, 128, 512], F16)   # normalized y^T, heads merged
            recip_dram = dpool.tile([B, 2 * NQC, 512], F32)

            # constants
            ident = singles.tile([128, 128], F16)
            make_identity(nc, ident)
            trimask = singles.tile([128, 128], F16)
            nc.gpsimd.memset(trimask, 1.0)
            nc.gpsimd.affine_select(
                out=trimask, in_=trimask,
                compare_op=mybir.AluOpType.is_ge,
                fill=0.0, base=0, pattern=[[1, 128]], channel_multiplier=-1,
            )
            # weights
            wq = singles.tile([128, 8, 128], F32R)
            wk = singles.tile([128, 8, 128], F32R)
            wv = singles.tile([128, 8, 128], F32R)
            wp = singles.tile([128, C], F16)
            nc.sync.dma_start(out=wq, in_=wq_d[:, :, :])
            nc.sync.dma_start(out=wk, in_=wk_d[:, :, :])
            nc.sync.dma_start(out=wv, in_=wv_d[:, :, :])
            nc.sync.dma_start(out=wp, in_=wp_d[:, :])
            # v_aug tiles for all (head, batch): [tokblk-part, blk, v|ones]
            v_aug = {}
            for h in range(HPC):
                for b in range(B):
                    va = singles.tile([128, 16, 65], F16, tag=f"va{h}{b}")
                    nc.vector.memset(va[:, :, 64:65], 1.0)
                    v_aug[(h, b)] = va

            QK_STRIDE_M = 2 * 64 * TOK   # elements per m (q or k) in qk_dram
            QK_STRIDE_H = 64 * TOK

            # ---------------- phase 1: qkvT + v transpose ----------------
            with (
                tc.tile_pool(name="xts", bufs=2) as xts,
                tc.tile_pool(name="evac", bufs=3) as evac,
                tc.tile_pool(name="ps1", bufs=2, space="PSUM") as ps1,
                tc.tile_pool(name="pvt", bufs=2, space="PSUM") as pvt,
            ):
                for ch in range(NCHUNK):
                    xt = xts.tile([128, 8, 512], F32R, tag="xt")
                    nc.sync.dma_start(
                        out=xt,
                        in_=_ap(xt_d[:, :], 512 * ch,
                                [[TOK, 128], [128 * TOK, 8], [1, 512]]),
                    )
                    for m, w in ((0, wq), (1, wk), (2, wv)):
                        ps = ps1.tile([128, 512], F32, tag="qkv")
                        for kb in range(8):
                            nc.tensor.matmul(
                                ps[:, :], w[:, kb, :], xt[:, kb, :],
                                start=(kb == 0), stop=(kb == 7),
                            )
                        t16 = evac.tile([128, 512], F16, tag=f"ev{m}")
                        nc.vector.tensor_copy(out=t16, in_=ps[:, :])
                        if m < 2:
                            nc.sync.dma_start(
                                out=_ap(qk_dram, m * QK_STRIDE_M + 512 * ch,
                                        [[QK_STRIDE_H, 2], [TOK, 64], [1, 512]]),
                                in_=t16,
                            )
                        else:
                            # transpose vT -> v, split heads along free dim
                            for sub in range(4):
                                tb = ch * 4 + sub
                                b, blk = tb // 16, tb % 16
                                pv = pvt.tile([128, 128], F16, tag="vt")
                                nc.tensor.transpose(
                                    pv[:, :], t16[:, sub * 128:(sub + 1) * 128],
                                    ident,
                                )
                                for h in range(HPC):
                                    nc.vector.tensor_copy(
                                        out=v_aug[(h, b)][:, blk, 0:64],
                                        in_=pv[:, h * 64:(h + 1) * 64],
                                    )

            # ---------------- phase 2+3: attention + proj ----------------
            with (
                tc.tile_pool(name="qk", bufs=2) as qkp,
                tc.tile_pool(name="pp", bufs=3) as ppool,
                tc.tile_pool(name="ysc", bufs=10) as yscp,
                tc.tile_pool(name="ysn", bufs=3) as ysnp,
                tc.tile_pool(name="proj", bufs=3) as projp,
                tc.tile_pool(name="sps", bufs=2, space="PSUM") as sps,
                tc.tile_pool(name="yps", bufs=1, space="PSUM") as yps,
                tc.tile_pool(name="ops", bufs=1, space="PSUM") as ops,
            ):
                for b in range(B):
                    ysc_tiles = {}
                    for h in range(HPC):
                        qT = qkp.tile([64, T], F16, tag="q")
                        kT = qkp.tile([64, T], F16, tag="k")
                        nc.sync.dma_start(
                            out=qT,
                            in_=_ap(qk_dram, h * QK_STRIDE_H + b * T,
                                    [[TOK, 64], [1, T]]),
                        )
                        nc.sync.dma_start(
                            out=kT,
                            in_=_ap(qk_dram, QK_STRIDE_M + h * QK_STRIDE_H + b * T,
                                    [[TOK, 64], [1, T]]),
                        )
                        va = v_aug[(h, b)]
                        for c in range(NQC):
                            n_j = 4 * (c + 1)
                            y_ps = yps.tile([65, 512], F32, tag="y")
                            first = True
                            for s0 in range(0, n_j, SUP):
                                jlist = list(range(s0, min(s0 + SUP, n_j)))
                                ln = len(jlist)
                                s_ps = sps.tile([128, SUP, 512], F32, tag="s")
                                for idx, j in enumerate(jlist):
                                    nc.tensor.matmul(
                                        s_ps[:, idx, :],
                                        kT[:, j * 128:(j + 1) * 128],
                                        qT[:, c * 512:(c + 1) * 512],
                                        start=True, stop=True,
                                    )
                                p_sb = ppool.tile([128, SUP, 512], F16, tag="p")
                                nc.scalar.activation(
                                    out=p_sb[:, 0:ln, :], in_=s_ps[:, 0:ln, :],
                                    func=mybir.ActivationFunctionType.Exp,
                                )
                                for idx, j in enumerate(jlist):
                                    if j >= 4 * c:
                                        off = (j - 4 * c) * 128
                                        if off > 0:
                                            nc.vector.memset(
                                                p_sb[:, idx, 0:off], 0.0)
                                        nc.vector.tensor_mul(
                                            p_sb[:, idx, off:off + 128],
                                            p_sb[:, idx, off:off + 128],
                                            trimask,
                                        )
                                for idx, j in enumerate(jlist):
                                    nc.tensor.matmul(
                                        y_ps[:, :], va[:, j, :], p_sb[:, idx, :],
                                        start=first,
                                        stop=(j == n_j - 1),
                                    )
                                    first = False
                            ysc = yscp.tile([65, 512], F32, tag="ysc")
                            nc.vector.tensor_copy(out=ysc, in_=y_ps[:, :])
                            nc.sync.dma_start(
                                out=_ap(y_stage,
                                        ((b * 2 + h) * NQC + c) * 65 * 512,
                                        [[512, 65], [1, 512]]),
                                in_=ysc,
                            )
                            ysc_tiles[(h, c)] = ysc

                    # denominators -> reciprocal -> normalize -> proj
                    dn = ysnp.tile([128, 8, 4], F32, tag="dn")
                    for hc in range(8):
                        nc.sync.dma_start(
                            out=dn[:, hc, :],
                            in_=_ap(y_stage,
                                    (b * 2 * NQC + hc) * 65 * 512 + 64 * 512,
                                    [[1, 128], [128, 4]]),
                        )
                    nc.vector.reciprocal(out=dn, in_=dn)
                    for hc in range(8):
                        nc.sync.dma_start(
                            out=_ap(recip_dram, (b * 8 + hc) * 512,
                                    [[1, 128], [128, 4]]),
                            in_=dn[:, hc, :],
                        )
                    for h in range(HPC):
                        for c in range(NQC):
                            rb = ysnp.tile([64, 512], F32, tag="rb")
                            nc.gpsimd.dma_start(
                                out=rb,
                                in_=_ap(recip_dram, (b * 8 + h * NQC + c) * 512,
                                        [[0, 64], [1, 512]]),
                            )
                            ysn = ysnp.tile([64, 512], F16, tag="ysn")
                            nc.vector.tensor_mul(
                                ysn, ysc_tiles[(h, c)][0:64, :], rb)
                            # y^T back through DRAM to regroup heads for proj
                            nc.sync.dma_start(
                                out=_ap(yn_dram,
                                        (b * NQC + c) * 128 * 512 + h * 64 * 512,
                                        [[512, 64], [1, 512]]),
                                in_=ysn,
                            )
                    # proj for this batch
                    for tb in range(16):
                        c, sub = tb // 4, tb % 4
                        yt = projp.tile([128, 128], F16, tag="yt")
                        nc.sync.dma_start(
                            out=yt,
                            in_=_ap(yn_dram,
                                    (b * NQC + c) * 128 * 512 + sub * 128,
                                    [[512, 128], [1, 128]]),
                        )
                        o_sb = projp.tile([128, C], F32, tag="osb")
                        for half in range(2):
                            o_ps = ops.tile([128, 512], F32, tag="o")
                            nc.tensor.matmul(
                                o_ps[:, :], yt[:, :],
                                wp[:, half * 512:(half + 1) * 512],
                                start=True, stop=True,
                            )
                            nc.vector.tensor_copy(
                                out=o_sb[:, half * 512:(half + 1) * 512],
                                in_=o_ps[:, :])
                        nc.sync.dma_start(
                            out=out_d[b * T + tb * 128: b * T + (tb + 1) * 128, :],
                            in_=o_sb,
                        )
            if debug:
                with tc.tile_pool(name="dbg", bufs=1) as dbgp:
                    nc.sync.dma_start(out=dbg_qk[:, :, :, :], in_=qk_dram[:, :, :, :])
                    nc.sync.dma_start(out=dbg_ys[:, :, :, :, :], in_=y_stage[:, :, :, :, :])
                    nc.sync.dma_start(out=dbg_rc[:, :, :], in_=recip_dram[:, :, :])
                    nc.sync.dma_start(out=dbg_yn[:, :, :, :], in_=yn_dram[:, :, :, :])
    nc.compile()
    return nc


def _host_inputs(x, w_attn, w_proj):
    x = np.asarray(x, dtype=np.float32)
    w_attn = np.asarray(w_attn, dtype=np.float32)
    w_proj = np.asarray(w_proj, dtype=np.float32)
    xT = np.ascontiguousarray(x.reshape(TOK, C).T)  # [C, TOK]
    in_maps = []
    for core in range(8):
        ch0 = core * HPC * HS  # first channel of this core's heads
        sl = slice(ch0, ch0 + 128)

        def lay(wslice):  # [1024, 128] -> [128(part=k in blk), 8(kblk), 128(ch)]
            return np.ascontiguousarray(
                wslice.reshape(8, 128, 128).transpose(1, 0, 2))

        wq = lay(w_attn[:, sl] * (HS ** -0.5))
        wk = lay(w_attn[:, C + ch0: C + ch0 + 128])
        wv = lay(w_attn[:, 2 * C + ch0: 2 * C + ch0 + 128])
        wp = np.ascontiguousarray(w_proj[sl, :]).astype(np.float16)
        in_maps.append({
            "xt": xT, "wq": wq, "wk": wk, "wv": wv, "wp": wp,
        })
    return in_maps


def kernel(x, w_attn, w_proj):
    import os
    if "nc" not in _CACHE:
        _CACHE["nc"] = build()
    nc = _CACHE["nc"]
    in_maps = _host_inputs(x, w_attn, w_proj)
    trace = os.environ.get("BASS_KERNEL_TRACE", "0") == "1"
    res = run_bass_kernel_spmd(nc, in_maps, core_ids=list(range(8)),
                               trace=trace)
    _CACHE["last_exec_time_ns"] = res.exec_time_ns
    parts = np.stack([res.results[c]["out"] for c in range(8)])
    out = parts.sum(axis=0, dtype=np.float64).astype(np.float32)
    return out.reshape(B, T, C)


# revision 12
# speedup vs baseline: 1.9720x; 1.9720x over previous
"""Causal self-attention (B=4, T=2048, C=1024, H=16) on 8 trn2 NeuronCores.

Strategy: tensor-parallel over heads (2 heads/core). Each core:
  phase 1: qkvT[ch, tok] = w_slice.T @ x.T   (fp32r matmuls, full x replicated)
           v transposed on PE to [tok, ch] layout, heads split along free dim
  phase 2: per (batch, head): S^T blocks = k.T-tiles @ q.T-chunks (fp16),
           exp on ACT, causal mask on DVE, PV matmuls accumulate
           y^T[hs+ones, qtok] in PSUM (denominator via ones column)
  phase 3: per batch: reciprocal of denominators (DVE), broadcast via DRAM,
           normalize y, c_proj matmuls (fp16), partial out[tok, C] fp32
Host sums the 8 partial outputs (tensor-parallel all-reduce done on host).

All SBUF/PSUM compute APs use partition offset 0 (nonzero offsets silently
read wrong partitions on this toolchain); head regrouping goes through DRAM.
"""
import sys
sys.path.insert(0, '/opt/trn_rl_repo')

import numpy as np

import concourse.bass as bass
import concourse.bacc as bacc_mod
import concourse.mybir as mybir
import concourse.tile as tile
from concourse.bass_utils import run_bass_kernel_spmd
from concourse.masks import make_identity

F32 = mybir.dt.float32
F32R = mybir.dt.float32r
F16 = mybir.dt.float16

B, T, C, H, HS = 4, 2048, 1024, 16, 64
HPC = H // 8            # heads per core = 2
TOK = B * T             # 8192
NCHUNK = TOK // 512     # 16 tok chunks for qkv
NQC = T // 512          # 4 qtok chunks per batch
SUP = 3                 # ktok blocks per exp supertile (3 psum banks)

_CACHE = {}


def _ap(t, offset, ap):
    return bass.AP(tensor=t.tensor, offset=t.offset + offset, ap=ap)


def build(debug=False):
    nc = bacc_mod.Bacc(target_bir_lowering=False, trn_type="TRN2")

    # fp32r inputs for the qkv projection (host pre-transposed / pre-sliced)
    xt_d = nc.dram_tensor("xt", [C, TOK], F16, kind="ExternalInput")
    wq_d = nc.dram_tensor("wq", [128, 8, 128], F16, kind="ExternalInput")
    wk_d = nc.dram_tensor("wk", [128, 8, 128], F16, kind="ExternalInput")
    wv_d = nc.dram_tensor("wv", [128, 8, 128], F16, kind="ExternalInput")
    wp_d = nc.dram_tensor("wp", [128, C], F16, kind="ExternalInput")
    out_d = nc.dram_tensor("out", [TOK, C], F16, kind="ExternalOutput")
    if debug:
        dbg_qk = nc.dram_tensor("dbg_qk", [2, 2, 64, TOK], F16, kind="ExternalOutput")
        dbg_ys = nc.dram_tensor("dbg_ys", [B, 2, NQC, 65, 512], F32, kind="ExternalOutput")
        dbg_rc = nc.dram_tensor("dbg_rc", [B, 2 * NQC, 512], F32, kind="ExternalOutput")
        dbg_yn = nc.dram_tensor("dbg_yn", [B, NQC, 4, 128, 128], F16, kind="ExternalOutput")

    with tile.TileContext(nc) as tc:
        with (
            tc.tile_pool(name="singles", bufs=1) as singles,
            tc.tile_pool(name="dram", bufs=1, space="DRAM") as dpool,
        ):
            # persistent DRAM scratch
            qk_dram = dpool.tile([2, 2, 64, TOK], F16)      # [qk, head, ch, tok]
            y_stage = dpool.tile([B, 2, NQC, 65, 512], F32)  # y^T + denom row 64
            yn_dram = dpool.tile([B, NQC, 4, 128, 128], F16)  # contiguous proj blocks
            recip_dram = dpool.tile([B, 2 * NQC, 512], F32)

            # constants
            ident = singles.tile([128, 128], F16)
            make_identity(nc, ident)
            trimask = singles.tile([128, 128], F16)
            nc.gpsimd.memset(trimask, 1.0)
            nc.gpsimd.affine_select(
                out=trimask, in_=trimask,
                compare_op=mybir.AluOpType.is_ge,
                fill=0.0, base=0, pattern=[[1, 128]], channel_multiplier=-1,
            )
            # weights
            wq = singles.tile([128, 8, 128], F16)
            wk = singles.tile([128, 8, 128], F16)
            wv = singles.tile([128, 8, 128], F16)
            wp = singles.tile([128, C], F16)
            nc.scalar.dma_start(out=wq, in_=wq_d[:, :, :])
            nc.scalar.dma_start(out=wk, in_=wk_d[:, :, :])
            nc.scalar.dma_start(out=wv, in_=wv_d[:, :, :])
            nc.sync.dma_start(out=wp, in_=wp_d[:, :])
            # v_aug tiles for all (head, batch): [tokblk-part, blk, v|ones]
            v_aug = {}
            for h in range(HPC):
                for b in range(B):
                    va = singles.tile([128, 16, 65], F16, tag=f"va{h}{b}")
                    nc.vector.memset(va[:, :, 64:65], 1.0)
                    v_aug[(h, b)] = va

            QK_STRIDE_M = 2 * 64 * TOK   # elements per m (q or k) in qk_dram
            QK_STRIDE_H = 64 * TOK

            # ---------------- phase 1: qkvT + v transpose ----------------
            with (
                tc.tile_pool(name="xts", bufs=2) as xts,
                tc.tile_pool(name="evac", bufs=3) as evac,
                tc.tile_pool(name="ps1", bufs=2, space="PSUM") as ps1,
                tc.tile_pool(name="pvt", bufs=2, space="PSUM") as pvt,
            ):
                for ch in range(NCHUNK):
                    xt = xts.tile([128, 8, 512], F16, tag="xt")
                    nc.scalar.dma_start(
                        out=xt,
                        in_=_ap(xt_d[:, :], 512 * ch,
                                [[TOK, 128], [128 * TOK, 8], [1, 512]]),
                    )
                    for m, w in ((0, wq), (1, wk), (2, wv)):
                        ps = ps1.tile([128, 512], F32, tag="qkv")
                        for kb in range(8):
                            nc.tensor.matmul(
                                ps[:, :], w[:, kb, :], xt[:, kb, :],
                                start=(kb == 0), stop=(kb == 7),
                            )
                        t16 = evac.tile([128, 512], F16, tag=f"ev{m}")
                        nc.vector.tensor_copy(out=t16, in_=ps[:, :])
                        if m < 2:
                            nc.scalar.dma_start(
                                out=_ap(qk_dram, m * QK_STRIDE_M + 512 * ch,
                                        [[QK_STRIDE_H, 2], [TOK, 64], [1, 512]]),
                                in_=t16,
                            )
                        else:
                            # transpose vT -> v, split heads along free dim
                            for sub in range(4):
                                tb = ch * 4 + sub
                                b, blk = tb // 16, tb % 16
                                pv = pvt.tile([128, 128], F16, tag="vt")
                                nc.tensor.transpose(
                                    pv[:, :], t16[:, sub * 128:(sub + 1) * 128],
                                    ident,
                                )
                                for h in range(HPC):
                                    nc.vector.tensor_copy(
                                        out=v_aug[(h, b)][:, blk, 0:64],
                                        in_=pv[:, h * 64:(h + 1) * 64],
                                    )

            # ---------------- phase 2+3: attention + proj ----------------
            with (
                tc.tile_pool(name="qk", bufs=2) as qkp,
                tc.tile_pool(name="pp", bufs=3) as ppool,
                tc.tile_pool(name="ysc", bufs=10) as yscp,
                tc.tile_pool(name="ysn", bufs=3) as ysnp,
                tc.tile_pool(name="proj", bufs=3) as projp,
                tc.tile_pool(name="sps", bufs=2, space="PSUM") as sps,
                tc.tile_pool(name="yps", bufs=1, space="PSUM") as yps,
                tc.tile_pool(name="ops", bufs=1, space="PSUM") as ops,
            ):
                for b in range(B):
                    ysc_tiles = {}
                    for h in range(HPC):
                        qT = qkp.tile([64, T], F16, tag="q")
                        kT = qkp.tile([64, T], F16, tag="k")
                        nc.sync.dma_start(
                            out=qT,
                            in_=_ap(qk_dram, h * QK_STRIDE_H + b * T,
                                    [[TOK, 64], [1, T]]),
                        )
                        nc.sync.dma_start(
                            out=kT,
                            in_=_ap(qk_dram, QK_STRIDE_M + h * QK_STRIDE_H + b * T,
                                    [[TOK, 64], [1, T]]),
                        )
                        va = v_aug[(h, b)]
                        for c in range(NQC):
                            n_j = 4 * (c + 1)
                            y_ps = yps.tile([65, 512], F32, tag="y")
                            first = True
                            for s0 in range(0, n_j, SUP):
                                jlist = list(range(s0, min(s0 + SUP, n_j)))
                                ln = len(jlist)
                                s_ps = sps.tile([128, SUP, 512], F32, tag="s")
                                for idx, j in enumerate(jlist):
                                    nc.tensor.matmul(
                                        s_ps[:, idx, :],
                                        kT[:, j * 128:(j + 1) * 128],
                                        qT[:, c * 512:(c + 1) * 512],
                                        start=True, stop=True,
                                    )
                                p_sb = ppool.tile([128, SUP, 512], F16, tag="p")
                                nc.scalar.activation(
                                    out=p_sb[:, 0:ln, :], in_=s_ps[:, 0:ln, :],
                                    func=mybir.ActivationFunctionType.Exp,
                                )
                                for idx, j in enumerate(jlist):
                                    if j >= 4 * c:
                                        off = (j - 4 * c) * 128
                                        if off > 0:
                                            nc.vector.memset(
                                                p_sb[:, idx, 0:off], 0.0)
                                        nc.vector.tensor_mul(
                                            p_sb[:, idx, off:off + 128],
                                            p_sb[:, idx, off:off + 128],
                                            trimask,
                                        )
                                for idx, j in enumerate(jlist):
                                    nc.tensor.matmul(
                                        y_ps[:, :], va[:, j, :], p_sb[:, idx, :],
                                        start=first,
                                        stop=(j == n_j - 1),
                                    )
                                    first = False
                            ysc = yscp.tile([65, 512], F32, tag="ysc")
                            nc.vector.tensor_copy(out=ysc, in_=y_ps[:, :])
                            nc.scalar.dma_start(
                                out=_ap(y_stage,
                                        ((b * 2 + h) * NQC + c) * 65 * 512,
                                        [[512, 65], [1, 512]]),
                                in_=ysc,
                            )
                            ysc_tiles[(h, c)] = ysc

                    # denominators -> reciprocal -> normalize -> proj
                    dn = ysnp.tile([128, 8, 4], F32, tag="dn")
                    for hc in range(8):
                        nc.sync.dma_start(
                            out=dn[:, hc, :],
                            in_=_ap(y_stage,
                                    (b * 2 * NQC + hc) * 65 * 512 + 64 * 512,
                                    [[1, 128], [128, 4]]),
                        )
                    nc.vector.reciprocal(out=dn, in_=dn)
                    for hc in range(8):
                        nc.sync.dma_start(
                            out=_ap(recip_dram, (b * 8 + hc) * 512,
                                    [[1, 128], [128, 4]]),
                            in_=dn[:, hc, :],
                        )
                    for h in range(HPC):
                        for c in range(NQC):
                            rb = ysnp.tile([64, 512], F32, tag="rb")
                            nc.gpsimd.dma_start(
                                out=rb,
                                in_=_ap(recip_dram, (b * 8 + h * NQC + c) * 512,
                                        [[0, 64], [1, 512]]),
                            )
                            ysn = ysnp.tile([64, 512], F16, tag="ysn")
                            nc.vector.tensor_mul(
                                ysn, ysc_tiles[(h, c)][0:64, :], rb)
                            # y^T back through DRAM to regroup heads for proj
                            for sub in range(4):
                                nc.sync.dma_start(
                                    out=_ap(yn_dram,
                                            ((b * NQC + c) * 4 + sub) * 16384
                                            + h * 64 * 128,
                                            [[128, 64], [1, 128]]),
                                    in_=ysn[:, sub * 128:(sub + 1) * 128],
                                )
                    # proj for this batch
                    for tb in range(16):
                        c, sub = tb // 4, tb % 4
                        yt = projp.tile([128, 128], F16, tag="yt")
                        nc.sync.dma_start(
                            out=yt,
                            in_=_ap(yn_dram,
                                    ((b * NQC + c) * 4 + sub) * 16384,
                                    [[128, 128], [1, 128]]),
                        )
                        o_sb = projp.tile([128, C], F16, tag="osb")
                        for half in range(2):
                            o_ps = ops.tile([128, 512], F32, tag="o")
                            nc.tensor.matmul(
                                o_ps[:, :], yt[:, :],
                                wp[:, half * 512:(half + 1) * 512],
                                start=True, stop=True,
                            )
                            nc.vector.tensor_copy(
                                out=o_sb[:, half * 512:(half + 1) * 512],
                                in_=o_ps[:, :])
                        nc.sync.dma_start(
                            out=out_d[b * T + tb * 128: b * T + (tb + 1) * 128, :],
                            in_=o_sb,
                        )
            if debug:
                with tc.tile_pool(name="dbg", bufs=1) as dbgp:
                    nc.sync.dma_start(out=dbg_qk[:, :, :, :], in_=qk_dram[:, :, :, :])
                    nc.sync.dma_start(out=dbg_ys[:, :, :, :, :], in_=y_stage[:, :, :, :, :])
                    nc.sync.dma_start(out=dbg_rc[:, :, :], in_=recip_dram[:, :, :])
                    nc.sync.dma_start(out=dbg_yn[:, :, :, :, :], in_=yn_dram[:, :, :, :, :])
    nc.compile()
    return nc


def _host_inputs(x, w_attn, w_proj):
    x = np.asarray(x, dtype=np.float32)
    w_attn = np.asarray(w_attn, dtype=np.float32)
    w_proj = np.asarray(w_proj, dtype=np.float32)
    xT = np.ascontiguousarray(x.reshape(TOK, C).T).astype(np.float16)  # [C, TOK]
    in_maps = []
    for core in range(8):
        ch0 = core * HPC * HS  # first channel of this core's heads
        sl = slice(ch0, ch0 + 128)

        def lay(wslice):  # [1024, 128] -> [128(part=k in blk), 8(kblk), 128(ch)]
            return np.ascontiguousarray(
                wslice.reshape(8, 128, 128).transpose(1, 0, 2))

        wq = lay(w_attn[:, sl] * (HS ** -0.5)).astype(np.float16)
        wk = lay(w_attn[:, C + ch0: C + ch0 + 128]).astype(np.float16)
        wv = lay(w_attn[:, 2 * C + ch0: 2 * C + ch0 + 128]).astype(np.float16)
        wp = np.ascontiguousarray(w_proj[sl, :]).astype(np.float16)
        in_maps.append({
            "xt": xT, "wq": wq, "wk": wk, "wv": wv, "wp": wp,
        })
    return in_maps


def kernel(x, w_attn, w_proj):
    import os
    if "nc" not in _CACHE:
        _CACHE["nc"] = build()
    nc = _CACHE["nc"]
    in_maps = _host_inputs(x, w_attn, w_proj)
    trace = os.environ.get("BASS_KERNEL_TRACE", "0") == "1"
    res = run_bass_kernel_spmd(nc, in_maps, core_ids=list(range(8)),
                               trace=trace)
    _CACHE["last_exec_time_ns"] = res.exec_time_ns
    parts = np.stack([res.results[c]["out"] for c in range(8)])
    out = parts.sum(axis=0, dtype=np.float32)
    return out.reshape(B, T, C)
